# revision 1
# baseline (speedup 1.0000x reference)
"""ContextQueryAttention (BiDAF trilinear attention) on 8 Trainium2 NeuronCores.

Full inputs:  c (32, 2048, 128) f32, q (32, 256, 128) f32, W0 (384,) f32
Full output:  (32, 2048, 512) f32 = concat([c, A, c*A, c*Bm], -1)

Sharding: pure data parallel — batch 32 is split 4-per-core across 8 cores;
every contraction is per-batch so there is no cross-core communication.

Per-batch math, with w1,w2,w3 = W0 split in 3, G = (c*w3) @ q^T:
    S[i,j] = c_i.w1 + q_j.w2 + G[i,j]
    S1 = softmax_j(S); S2 = softmax_i(S)
    A = S1 @ q ; T = S2^T @ c ; Bm = S1 @ T
Softmax normalization is linear in the contractions, so S is never
normalized on-chip.  With cw1[i] = c_i.w1 and g[j] = exp(q_j.w2):
    F_nat = exp(G + cw1)   natural [i,j] layout  (g factor cancels in T)
    F_t   = exp(G^T)       [j,i] layout          (exp(cw1) cancels in A/Bm)
    [UT|s'][j] = sum_i F_nat[i,j] * [c_i | 1]          -> T = UT/s'
    [UA|UBm|r][i] = sum_j F_t[j,i] * [g*q | g*T | g]   -> A = UA/r, Bm = UBm/r
"""

from contextlib import ExitStack

import numpy as np

import concourse.bacc as bacc
import concourse.bass as bass
import concourse.mybir as mybir
import concourse.tile as tile
from concourse.bass_utils import run_bass_kernel_spmd

F32 = mybir.dt.float32
BF16 = mybir.dt.bfloat16
P = 128

N_CORES = 8
B, LC, LQ, D = 32, 2048, 256, 128
BC = B // N_CORES


def _build_nc(BC=BC, LC=2048, LQ=256, D=128):
    NT = LC // P          # i-chunks
    NJ = LQ // P          # j-halves
    NPAIR = NT // 2       # paired i-chunks for the A/Bm epilogue
    OGRP = 4              # pairs per output staging tile / store

    nc = bacc.Bacc("TRN2", target_bir_lowering=False, debug=False)
    c_d = nc.dram_tensor("c", [BC, LC, D], F32, kind="ExternalInput").ap()
    q_d = nc.dram_tensor("q", [BC, LQ, D], F32, kind="ExternalInput").ap()
    w_d = nc.dram_tensor("W0", [3 * D], F32, kind="ExternalInput").ap()
    o_d = nc.dram_tensor("out", [BC, LC, 4 * D], F32, kind="ExternalOutput").ap()

    with tile.TileContext(nc) as tc, ExitStack() as ctx:
        singles = ctx.enter_context(tc.tile_pool(name="singles", bufs=1))
        cpool = ctx.enter_context(tc.tile_pool(name="cpool", bufs=3))
        qpool = ctx.enter_context(tc.tile_pool(name="qpool", bufs=3))
        fpool = ctx.enter_context(tc.tile_pool(name="fpool", bufs=2))
        ftpool = ctx.enter_context(tc.tile_pool(name="ftpool", bufs=2))
        opool = ctx.enter_context(tc.tile_pool(name="opool", bufs=3))
        small = ctx.enter_context(tc.tile_pool(name="small", bufs=6))
        psBig = ctx.enter_context(tc.tile_pool(name="psBig", bufs=3, space="PSUM"))
        psUT = ctx.enter_context(tc.tile_pool(name="psUT", bufs=2, space="PSUM"))

        # ---- one-time: weight vectors as per-partition columns [128, 3] ----
        w_sb = singles.tile([P, 3], F32)
        nc.gpsimd.dma_start(out=w_sb, in_=w_d.rearrange("(k p) -> p k", p=P))
        w_bf = singles.tile([P, 3], BF16)
        nc.vector.tensor_copy(out=w_bf, in_=w_sb)

        def emit_prep(b):
            """Loads, casts, transposes, and the q-side prep for batch b."""
            T = {}
            cn32 = cpool.tile([P, NT, D], F32, tag="cn32", name=f"cn32_{b}")
            nc.sync.dma_start(out=cn32, in_=c_d[b].rearrange("(t p) d -> p t d", p=P))
            # the c block of the output is an exact copy: one store per batch
            nc.sync.dma_start(
                out=o_d[b, :, 0:D].rearrange("(t p) d -> p t d", p=P), in_=cn32
            )
            cn16 = cpool.tile([P, NT, D], BF16, tag="cn16", name=f"cn16_{b}")
            nc.vector.tensor_copy(out=cn16, in_=cn32)
            # [c | 1] (129 wide) for the UT matmul rhs
            cne = cpool.tile([P, NT, D + 1], BF16, tag="cne", name=f"cne_{b}")
            nc.vector.tensor_copy(out=cne[:, :, 0:D], in_=cn32)
            nc.gpsimd.memset(cne[:, :, D : D + 1], 1.0)
            # c^T (bf16): ct16[d, t, ii] = c[t*128+ii, d]
            ct16 = cpool.tile([P, NT, P], BF16, tag="ct16", name=f"ct16_{b}")
            nc.sync.dma_start_transpose(
                out=ct16, in_=cn16.rearrange("p t d -> p (t d)")
            )

            qn32 = qpool.tile([P, NJ, D], F32, tag="qn32", name=f"qn32_{b}")
            nc.scalar.dma_start(
                out=qn32, in_=q_d[b].rearrange("(h p) d -> p h d", p=P)
            )
            qn16 = qpool.tile([P, NJ, D], BF16, tag="qn16", name=f"qn16_{b}")
            nc.vector.tensor_copy(out=qn16, in_=qn32)
            # q^T: qt16[d, h, jj] = q[h*128+jj, d]
            qt16 = qpool.tile([P, NJ, P], BF16, tag="qt16", name=f"qt16_{b}")
            nc.scalar.dma_start_transpose(
                out=qt16, in_=qn16.rearrange("p h d -> p (h d)")
            )

            # rhsG = [w3*q^T | w1]  (d on partitions, 257 wide)
            rhsG = qpool.tile([P, LQ + 1], BF16, tag="rhsG", name=f"rhsG_{b}")
            nc.vector.tensor_scalar_mul(
                out=rhsG[:, 0:LQ],
                in0=qt16.rearrange("p h j -> p (h j)"),
                scalar1=w_sb[:, 2:3],
            )
            nc.vector.tensor_copy(out=rhsG[:, LQ : LQ + 1], in_=w_bf[:, 0:1])

            # g = exp(q.w2) per j  (two [128,1] halves)
            qw2_ps = psUT.tile([P, NJ], F32, tag="ut", name=f"qw2_{b}")
            for h in range(NJ):
                nc.tensor.matmul(
                    out=qw2_ps[:, h : h + 1], lhsT=qt16[:, h, :], rhs=w_bf[:, 1:2]
                )
            g_sb = small.tile([P, NJ], F32, tag="g", name=f"g_{b}")
            nc.scalar.activation(
                out=g_sb, in_=qw2_ps, func=mybir.ActivationFunctionType.Exp
            )
            g_bf = small.tile([P, NJ], BF16, tag="gbf", name=f"gbf_{b}")
            nc.vector.tensor_copy(out=g_bf, in_=g_sb)

            # rhs for the A/Bm matmul: [g*q | g*T | g]; T~ filled later.
            qs16 = qpool.tile([P, NJ, 2 * D + 1], BF16, tag="qs16", name=f"qs16_{b}")
            for h in range(NJ):
                nc.vector.tensor_scalar_mul(
                    out=qs16[:, h, 0:D], in0=qn16[:, h, :], scalar1=g_sb[:, h : h + 1]
                )
                nc.vector.tensor_copy(
                    out=qs16[:, h, 2 * D : 2 * D + 1], in_=g_bf[:, h : h + 1]
                )
            T.update(
                cn32=cn32, cn16=cn16, cne=cne, ct16=ct16, rhsG=rhsG,
                g_sb=g_sb, qs16=qs16,
            )
            return T

        def emit_natural(b, Tt):
            """G matmuls, exp, UT accumulation, F^T transpose, T~ fill."""
            fn_all = fpool.tile([P, NT, LQ], BF16, tag="fn", name=f"fn_{b}")
            ut_ps = [
                psUT.tile([P, D + 1], F32, tag="ut", name=f"ut{h}_{b}")
                for h in range(NJ)
            ]
            for t in range(NT):
                gps = psBig.tile(
                    [P, LQ + 1], F32, tag="big", padded_shape=[P, 2 * 512],
                    name=f"gps_{b}_{t}",
                )
                nc.tensor.matmul(out=gps, lhsT=Tt["ct16"][:, t, :], rhs=Tt["rhsG"])
                # stage the cw1 column to SBUF (DVE; off the ACT chain)
                cw1_sb = small.tile([P, 1], F32, tag="cw1", name=f"cw1_{b}_{t}")
                nc.vector.tensor_copy(out=cw1_sb, in_=gps[:, LQ : LQ + 1])
                nc.scalar.activation(
                    out=fn_all[:, t, :],
                    in_=gps[:, 0:LQ],
                    func=mybir.ActivationFunctionType.Exp,
                    bias=cw1_sb,
                )
                for h in range(NJ):
                    nc.tensor.matmul(
                        out=ut_ps[h],
                        lhsT=fn_all[:, t, h * P : (h + 1) * P],
                        rhs=Tt["cne"][:, t, :],
                        start=(t == 0),
                        stop=(t == NT - 1),
                    )

            # F^T via two half-batch DMA transposes
            # ftr[j%128, 2t+jh, ii] = F[t*128+ii, j]
            ftr = ftpool.tile([P, 2 * NT, P], BF16, tag="ftr", name=f"ftr_{b}")
            for half in range(2):
                nc.sync.dma_start_transpose(
                    out=ftr[:, half * NT : (half + 1) * NT, :],
                    in_=fn_all[
                        :, half * (NT // 2) : (half + 1) * (NT // 2), :
                    ].rearrange("p t j -> p (t j)"),
                )

            # T~ = g * UT / s'  (into qs16 cols 128:256)
            for h in range(NJ):
                recip_s = small.tile([P, 1], F32, tag="rs", name=f"rs_{b}_{h}")
                nc.vector.reciprocal(out=recip_s, in_=ut_ps[h][:, D : D + 1])
                scale_j = small.tile([P, 1], F32, tag="sj", name=f"sj_{b}_{h}")
                nc.vector.tensor_mul(
                    out=scale_j, in0=Tt["g_sb"][:, h : h + 1], in1=recip_s
                )
                nc.vector.tensor_scalar_mul(
                    out=Tt["qs16"][:, h, D : 2 * D],
                    in0=ut_ps[h][:, 0:D],
                    scalar1=scale_j,
                )
            Tt["ftr"] = ftr

        def emit_ab(b, Tt):
            """A/Bm matmuls + epilogue, two i-chunks at a time."""
            ftr, qs16, cn32 = Tt["ftr"], Tt["qs16"], Tt["cn32"]
            for og in range(NPAIR // OGRP):
                out_sb = opool.tile(
                    [P, 2 * OGRP, 3 * D], F32, tag="osb", name=f"osb_{b}_{og}"
                )
                for pp in range(OGRP):
                    tp = og * OGRP + pp
                    ab_ps = psBig.tile(
                        [P, 2, 2 * D + 1], F32, tag="big",
                        padded_shape=[P, 2, 512], name=f"ab_{b}_{tp}",
                    )
                    for k in range(2):
                        t = 2 * tp + k
                        for h in range(NJ):
                            nc.tensor.matmul(
                                out=ab_ps[:, k, :],
                                lhsT=ftr[:, 2 * t + h, :],
                                rhs=qs16[:, h, :],
                                start=(h == 0),
                                stop=(h == NJ - 1),
                            )
                    # rr[p, k] = 1 / r
                    rr = small.tile([P, 2], F32, tag="rr", name=f"rr_{b}_{tp}")
                    nc.vector.reciprocal(out=rr, in_=ab_ps[:, :, 2 * D])
                    rr_b = bass.AP(
                        tensor=rr.tensor,
                        offset=rr.offset,
                        ap=[rr.ap[0], rr.ap[1], [0, 2 * D]],
                    )
                    osl = out_sb[:, 2 * pp : 2 * pp + 2, :]
                    # [A | Bm] = [UA | UBm] * rr   (rr broadcast along free)
                    nc.vector.tensor_mul(
                        out=osl[:, :, 0 : 2 * D],
                        in0=ab_ps[:, :, 0 : 2 * D],
                        in1=rr_b,
                    )
                    cc = cn32[:, 2 * tp : 2 * tp + 2, :]
                    # c*Bm (reads the Bm slot), then c*A overwrites the Bm slot
                    nc.gpsimd.tensor_mul(
                        out=osl[:, :, 2 * D : 3 * D],
                        in0=osl[:, :, D : 2 * D],
                        in1=cc,
                    )
                    nc.gpsimd.tensor_mul(
                        out=osl[:, :, D : 2 * D], in0=osl[:, :, 0:D], in1=cc
                    )
                nc.sync.dma_start(
                    out=o_d[
                        b, og * OGRP * 2 * P : (og + 1) * OGRP * 2 * P, D : 4 * D
                    ].rearrange("(k p) x -> p k x", p=P),
                    in_=out_sb,
                )

        tiles = {0: emit_prep(0)}
        for b in range(BC):
            emit_natural(b, tiles[b])
            if b + 1 < BC:
                tiles[b + 1] = emit_prep(b + 1)
            emit_ab(b, tiles[b])
            del tiles[b]

    nc.finalize()
    return nc


_NC_CACHE = None


def _get_nc():
    global _NC_CACHE
    if _NC_CACHE is None:
        _NC_CACHE = _build_nc()
    return _NC_CACHE


def run(c, q, W0, trace=False):
    c = np.ascontiguousarray(np.asarray(c, dtype=np.float32))
    q = np.ascontiguousarray(np.asarray(q, dtype=np.float32))
    W0 = np.ascontiguousarray(np.asarray(W0, dtype=np.float32))
    assert c.shape == (B, LC, D) and q.shape == (B, LQ, D) and W0.shape == (3 * D,)

    nc = _get_nc()
    in_maps = [
        {"c": c[k * BC : (k + 1) * BC], "q": q[k * BC : (k + 1) * BC], "W0": W0}
        for k in range(N_CORES)
    ]
    res = run_bass_kernel_spmd(nc, in_maps, core_ids=list(range(N_CORES)), trace=trace)
    out = np.concatenate([res.results[k]["out"] for k in range(N_CORES)], axis=0)
    return out, res


def kernel(c, q, W0):
    out, _ = run(c, q, W0, trace=False)
    return out



# revision 14
# speedup vs baseline: 1.1644x; 1.1644x over previous
"""ContextQueryAttention (BiDAF trilinear attention) on 8 Trainium2 NeuronCores.

Full inputs:  c (32, 2048, 128) f32, q (32, 256, 128) f32, W0 (384,) f32
Full output:  (32, 2048, 512) f32 = concat([c, A, c*A, c*Bm], -1)

Sharding: pure data parallel - batch 32 is split 4-per-core across 8 cores;
every contraction is per-batch so there is no cross-core communication.

Per-batch math, with w1,w2,w3 = W0 split in 3, G = (c*w3) @ q^T:
    S[i,j] = c_i.w1 + q_j.w2 + G[i,j]
    S1 = softmax_j(S); S2 = softmax_i(S)
    A = S1 @ q ; T = S2^T @ c ; Bm = S1 @ T
Softmax normalization is linear in the contractions, so S is never
normalized on-chip.  With ecw1[i] = exp(c_i.w1) and g[j] = exp(q_j.w2):
    F   = exp(G)        natural [i,j] layout (cw1/qw2 factors folded out)
    F_t = exp(G^T)      [j,i] layout, computed DIRECTLY from a second PE
                        matmul (lhsT = w3*q^T, rhs = c^T) - no DMA transpose
    [UT|s'][j] = sum_i F[i,j] * ecw1[i] * [c_i | 1]     -> T = UT/s'
    [UA|UBm|r][i] = sum_j F_t[j,i] * [g*q | g*T | g]    -> A = UA/r, Bm = UBm/r

Row mapping: context row i lives on partition i//16, slot i%16 (k-chunk k
= i%16 spans all 128 partitions).  This makes the c load 8KB-contiguous
per partition and the full-row output store 8KB-contiguous per partition
(4 rows x 2048B), instead of 512B strided lines.
"""

from contextlib import ExitStack

import numpy as np

import concourse.bacc as bacc
import concourse.bass as bass
import concourse.mybir as mybir
import concourse.tile as tile
from concourse.bass_utils import run_bass_kernel_spmd

F32 = mybir.dt.float32
BF16 = mybir.dt.bfloat16
P = 128

N_CORES = 8
B, LC, LQ, D = 32, 2048, 256, 128
BC = B // N_CORES

EXP = mybir.ActivationFunctionType.Exp


def _build_nc(BC=BC, LC=2048, LQ=256, D=128):
    NK = LC // P          # 16 i-chunks (chunk k = rows {p*16+k})
    NJ = LQ // P          # 2 j-halves
    NPAIR = NK // 2       # 8 paired i-chunks in the A/Bm epilogue
    NG = 4                # store groups per batch (4 k-rows each)

    nc = bacc.Bacc("TRN2", target_bir_lowering=False, debug=False)
    c_d = nc.dram_tensor("c", [BC, LC, D], F32, kind="ExternalInput").ap()
    q_d = nc.dram_tensor("q", [BC, LQ, D], F32, kind="ExternalInput").ap()
    w_d = nc.dram_tensor("W0", [3 * D], F32, kind="ExternalInput").ap()
    o_d = nc.dram_tensor("out", [BC, LC, 4 * D], F32, kind="ExternalOutput").ap()

    with tile.TileContext(nc) as tc, ExitStack() as ctx:
        singles = ctx.enter_context(tc.tile_pool(name="singles", bufs=1))
        cpool = ctx.enter_context(tc.tile_pool(name="cpool", bufs=3))
        qpool = ctx.enter_context(tc.tile_pool(name="qpool", bufs=3))
        fnpool = ctx.enter_context(tc.tile_pool(name="fnpool", bufs=2))
        ftpool = ctx.enter_context(tc.tile_pool(name="ftpool", bufs=2))
        opool = ctx.enter_context(tc.tile_pool(name="opool", bufs=3))
        small = ctx.enter_context(tc.tile_pool(name="small", bufs=6))
        # PSUM: 8 banks = psBig 3x2 + psUT 2x1
        psBig = ctx.enter_context(tc.tile_pool(name="psBig", bufs=3, space="PSUM"))
        psUT = ctx.enter_context(tc.tile_pool(name="psUT", bufs=2, space="PSUM"))

        # ---- one-time: weight vectors as per-partition columns [128, 3] ----
        w_sb = singles.tile([P, 3], F32)
        nc.gpsimd.dma_start(out=w_sb, in_=w_d.rearrange("(k p) -> p k", p=P))
        w_bf = singles.tile([P, 3], BF16)
        nc.vector.tensor_copy(out=w_bf, in_=w_sb)

        def emit_prep(b):
            """Loads, casts, transposes, cw1/qw2/g, cne, q-side prep."""
            T = {}
            # c natural: partition p holds rows {16p+k}, 8KB contiguous load
            cn32 = cpool.tile([P, NK, D], F32, tag="cn32", name=f"cn32_{b}")
            nc.sync.dma_start(out=cn32, in_=c_d[b].rearrange("(p k) d -> p k d", k=NK))
            cn16 = cpool.tile([P, NK, D], BF16, tag="cn16", name=f"cn16_{b}")
            nc.vector.tensor_copy(out=cn16, in_=cn32)
            # c^T (bf16): ct16[d, k, p] = c[16p+k, d]
            ct16 = cpool.tile([P, NK, P], BF16, tag="ct16", name=f"ct16_{b}")
            nc.sync.dma_start_transpose(out=ct16, in_=cn16.rearrange("p k d -> p (k d)"))

            qn32 = qpool.tile([P, NJ, D], F32, tag="qn32", name=f"qn32_{b}")
            nc.scalar.dma_start(out=qn32, in_=q_d[b].rearrange("(h p) d -> p h d", p=P))
            qn16 = qpool.tile([P, NJ, D], BF16, tag="qn16", name=f"qn16_{b}")
            nc.vector.tensor_copy(out=qn16, in_=qn32)
            # q^T: qt16[d, h, jj] = q[h*128+jj, d]
            qt16 = qpool.tile([P, NJ, P], BF16, tag="qt16", name=f"qt16_{b}")
            nc.scalar.dma_start_transpose(out=qt16, in_=qn16.rearrange("p h d -> p (h d)"))

            # rhsG = w3*q^T  (d on partitions, 256 wide)
            rhsG = qpool.tile([P, LQ], BF16, tag="rhsG", name=f"rhsG_{b}")
            nc.vector.tensor_scalar_mul(
                out=rhsG,
                in0=qt16.rearrange("p h j -> p (h j)"),
                scalar1=w_sb[:, 2:3],
            )

            # prep_ps = [cw1 (16 cols) | qw2 (2 cols)]
            prep_ps = psBig.tile(
                [P, NK + NJ], F32, tag="big", padded_shape=[P, 1024],
                name=f"prep_{b}",
            )
            for k in range(NK):
                nc.tensor.matmul(
                    out=prep_ps[:, k : k + 1], lhsT=ct16[:, k, :], rhs=w_bf[:, 0:1]
                )
            for h in range(NJ):
                nc.tensor.matmul(
                    out=prep_ps[:, NK + h : NK + h + 1],
                    lhsT=qt16[:, h, :],
                    rhs=w_bf[:, 1:2],
                )
            # pexp = [exp(cw1) | exp(qw2)] = [ecw1 | g]
            pexp = small.tile([P, NK + NJ], F32, tag="pexp", name=f"pexp_{b}")
            nc.scalar.activation(out=pexp, in_=prep_ps, func=EXP)

            # cne = [ecw1*c | ecw1]  (129 wide, bf16) for the UT matmul rhs
            cne = cpool.tile([P, NK, D + 1], BF16, tag="cne", name=f"cne_{b}")
            ecw1_b = bass.AP(
                tensor=pexp.tensor,
                offset=pexp.offset,
                ap=[pexp.ap[0], [pexp.ap[1][0], NK], [0, D]],
            )
            nc.vector.tensor_mul(out=cne[:, :, 0:D], in0=cn32, in1=ecw1_b)
            nc.vector.tensor_copy(out=cne[:, :, D], in_=pexp[:, 0:NK])

            g_bf = small.tile([P, NJ], BF16, tag="gbf", name=f"gbf_{b}")
            nc.vector.tensor_copy(out=g_bf, in_=pexp[:, NK : NK + NJ])

            # rhs for the A/Bm matmul: [g*q | g*T | g]; T~ filled later.
            qs16 = qpool.tile([P, NJ, 2 * D + 1], BF16, tag="qs16", name=f"qs16_{b}")
            for h in range(NJ):
                nc.vector.tensor_scalar_mul(
                    out=qs16[:, h, 0:D],
                    in0=qn16[:, h, :],
                    scalar1=pexp[:, NK + h : NK + h + 1],
                )
            nc.vector.tensor_copy(out=qs16[:, :, 2 * D], in_=g_bf)
            T.update(
                cn32=cn32, ct16=ct16, cne=cne, rhsG=rhsG, pexp=pexp, qs16=qs16,
            )
            return T

        def emit_natural(b, Tt):
            """G matmuls + exp + UT accumulation; F_t via G^T matmuls; T~."""
            ct16, rhsG, cne = Tt["ct16"], Tt["rhsG"], Tt["cne"]
            fn_all = fnpool.tile([P, NK, LQ], BF16, tag="fn", name=f"fn_{b}")
            # [UT | s'] for both halves in ONE psum bank: row h cols 0:129
            # one accumulation group per PSUM bank (interleaved groups in one
            # bank's zero region are illegal)
            ut_ps = [
                psUT.tile([P, D + 1], F32, tag="ut", name=f"ut{h}_{b}")
                for h in range(NJ)
            ]
            for pair in range(NPAIR):
                gps = psBig.tile(
                    [P, 2, 512], F32, tag="big", name=f"gps_{b}_{pair}"
                )
                for kk in range(2):
                    k = 2 * pair + kk
                    nc.tensor.matmul(
                        out=gps[:, kk, 0:LQ], lhsT=ct16[:, k, :], rhs=rhsG
                    )
                for kk in range(2):
                    k = 2 * pair + kk
                    nc.scalar.activation(
                        out=fn_all[:, k, :], in_=gps[:, kk, 0:LQ], func=EXP
                    )
                for kk in range(2):
                    k = 2 * pair + kk
                    for h in range(NJ):
                        nc.tensor.matmul(
                            out=ut_ps[h],
                            lhsT=fn_all[:, k, h * P : (h + 1) * P],
                            rhs=cne[:, k, :],
                            start=(k == 0),
                            stop=(k == NK - 1),
                        )

            # F_t = exp(G^T) directly: lhsT = w3*q^T half, rhs = c^T chunks
            ft_all = ftpool.tile([P, NJ, NK, P], BF16, tag="ft", name=f"ft_{b}")
            for h in range(NJ):
                for cc in range(2):
                    ft_ps = psBig.tile(
                        [P, 2, 512], F32, tag="big", name=f"ftps_{b}_{h}_{cc}"
                    )
                    for s in range(2):
                        k0 = cc * 8 + s * 4
                        nc.tensor.matmul(
                            out=ft_ps[:, s, :],
                            lhsT=rhsG[:, h * P : (h + 1) * P],
                            rhs=ct16[:, k0 : k0 + 4, :].rearrange(
                                "p a b -> p (a b)"
                            ),
                        )
                    # one exp per PSUM bank: a single read spanning both
                    # matmuls' banks races the first matmul on HW
                    for s in range(2):
                        k0 = cc * 8 + s * 4
                        nc.scalar.activation(
                            out=ft_all[:, h, k0 : k0 + 4, :].rearrange(
                                "p a b -> p (a b)"
                            ),
                            in_=ft_ps[:, s, :],
                            func=EXP,
                        )

            # T~ = g * UT / s'  (into qs16 cols 128:256)
            rs = small.tile([P, NJ], F32, tag="rs", name=f"rs_{b}")
            for h in range(NJ):
                nc.vector.reciprocal(
                    out=rs[:, h : h + 1], in_=ut_ps[h][:, D : D + 1]
                )
            scale_j = small.tile([P, NJ], F32, tag="sj", name=f"sj_{b}")
            nc.vector.tensor_mul(
                out=scale_j, in0=Tt["pexp"][:, NK : NK + NJ], in1=rs
            )
            for h in range(NJ):
                nc.vector.tensor_scalar_mul(
                    out=Tt["qs16"][:, h, D : 2 * D],
                    in0=ut_ps[h][:, 0:D],
                    scalar1=scale_j[:, h : h + 1],
                )
            Tt["ft"] = ft_all

        def emit_ab(b, Tt):
            """A/Bm matmuls + epilogue, one store group (4 k-rows) at a time."""
            ft, qs16, cn32 = Tt["ft"], Tt["qs16"], Tt["cn32"]
            for og in range(NG):
                out_sb = opool.tile(
                    [P, 4, 4 * D], F32, tag="osb", name=f"osb_{b}_{og}"
                )
                nc.vector.tensor_copy(
                    out=out_sb[:, :, 0:D], in_=cn32[:, og * 4 : og * 4 + 4, :]
                )
                for pp in range(2):
                    tp = og * 2 + pp
                    ab_ps = psBig.tile(
                        [P, 2, 512], F32, tag="big", name=f"ab_{b}_{tp}"
                    )
                    for kk in range(2):
                        k = 2 * tp + kk
                        for h in range(NJ):
                            nc.tensor.matmul(
                                out=ab_ps[:, kk, 0 : 2 * D + 1],
                                lhsT=ft[:, h, k, :],
                                rhs=qs16[:, h, :],
                                start=(h == 0),
                                stop=(h == NJ - 1),
                            )
                    # rr[p, kk] = 1 / r
                    rr = small.tile([P, 2], F32, tag="rr", name=f"rr_{b}_{tp}")
                    nc.vector.reciprocal(
                        out=rr,
                        in_=ab_ps[:, :, 2 * D : 2 * D + 1].rearrange(
                            "p a b -> p (a b)"
                        ),
                    )
                    rr_b = bass.AP(
                        tensor=rr.tensor,
                        offset=rr.offset,
                        ap=[rr.ap[0], rr.ap[1], [0, 2 * D]],
                    )
                    # [A | Bm] = [UA | UBm] * rr -> staging cols [128:384]
                    nc.vector.tensor_mul(
                        out=out_sb[:, 2 * pp : 2 * pp + 2, D : 3 * D],
                        in0=ab_ps[:, :, 0 : 2 * D],
                        in1=rr_b,
                    )
                cc = cn32[:, og * 4 : og * 4 + 4, :]
                # c*Bm (reads Bm from the cA slot), then c*A overwrites that slot
                nc.gpsimd.tensor_mul(
                    out=out_sb[:, :, 3 * D : 4 * D],
                    in0=out_sb[:, :, 2 * D : 3 * D],
                    in1=cc,
                )
                nc.gpsimd.tensor_mul(
                    out=out_sb[:, :, 2 * D : 3 * D],
                    in0=out_sb[:, :, D : 2 * D],
                    in1=cc,
                )
                nc.sync.dma_start(
                    out=o_d[b].rearrange("(p k) x -> p k x", k=NK)[
                        :, og * 4 : og * 4 + 4, :
                    ],
                    in_=out_sb,
                )

        tiles = {0: emit_prep(0)}
        for b in range(BC):
            emit_natural(b, tiles[b])
            if b + 1 < BC:
                tiles[b + 1] = emit_prep(b + 1)
            emit_ab(b, tiles[b])
            del tiles[b]

    nc.finalize()
    return nc


_NC_CACHE = None


def _get_nc():
    global _NC_CACHE
    if _NC_CACHE is None:
        _NC_CACHE = _build_nc()
    return _NC_CACHE


def run(c, q, W0, trace=False):
    c = np.ascontiguousarray(np.asarray(c, dtype=np.float32))
    q = np.ascontiguousarray(np.asarray(q, dtype=np.float32))
    W0 = np.ascontiguousarray(np.asarray(W0, dtype=np.float32))
    assert c.shape == (B, LC, D) and q.shape == (B, LQ, D) and W0.shape == (3 * D,)

    nc = _get_nc()
    in_maps = [
        {"c": c[k * BC : (k + 1) * BC], "q": q[k * BC : (k + 1) * BC], "W0": W0}
        for k in range(N_CORES)
    ]
    res = run_bass_kernel_spmd(nc, in_maps, core_ids=list(range(N_CORES)), trace=trace)
    out = np.concatenate([res.results[k]["out"] for k in range(N_CORES)], axis=0)
    return out, res


def kernel(c, q, W0):
    out, _ = run(c, q, W0, trace=False)
    return out
